# revision 1
# baseline (speedup 1.0000x reference)
"""Trainium2 Bass kernel for linear attention over external memory.

Computes out = x @ (keys^T @ vals) for
  x [4, 2048, 1024] f32, keys/vals [65536, 1024] f32.

Sharding across 8 NeuronCores: keys/vals sharded along the memory dim M
(8192 rows per core); each core computes a partial kv = keys_s^T @ vals_s,
AllReduces kv in fp16 (split in two column halves), then computes its
token shard of x @ kv (x sharded by token, 1024 rows per core).

Stage 1 runs in float32r (TF32-like, full PE rate for moving dim >= 256)
directly on the DMA'd f32 data.  kv is accumulated in PSUM per group of
8 k-chunks and drained into an fp16 SBUF accumulator.

Tail restructure: the last W=12 chunks are processed one column half at
a time.  While they stream, their keys (full) and vals' second half are
cast to an fp16 SBUF hold; the h=0 pass runs from f32r staging, so the
h=0 AllReduce fires ~30us before stage-1 ends and overlaps the h=1 pass
(which runs from the fp16 hold).  The h=1 AllReduce then overlaps the x
transposes and stage 2 on the h=0 columns.  Dummy matmul chains bridge
the PE-idle AllReduce waits so HAM doesn't re-throttle the PE clock
before stage 2.
"""

import numpy as np

# Problem shapes (hardcoded per contract).
B, S, D = 4, 2048, 1024
M = 65536
NCORES = 8
P = 128
T = (B * S) // NCORES          # 1024 tokens per core
KM = M // NCORES               # 8192 memory rows per core
NC_ = KM // P                  # 64 k-chunks
G = 8                          # chunks per PSUM accumulation group
DB = D // P                    # 8 d-blocks
HALF = D // 2                  # 512
TCH = T // P                   # 8 token chunks
W = 12                         # tail chunks (h-split, fp16 hold)
FRONT = NC_ - W                # 52
FRONT_GROUPS = [4, 8, 8, 8, 8, 8, 8]
assert sum(FRONT_GROUPS) == FRONT

_CACHE = {}


def _build_nc():
    import concourse.bacc as bacc
    import concourse.tile as tile
    from concourse import mybir
    from concourse.masks import make_identity

    f32 = mybir.dt.float32
    f32r = mybir.dt.float32r
    f16 = mybir.dt.float16
    ACT_COPY = mybir.ActivationFunctionType.Copy
    ADD = mybir.AluOpType.add

    nc = bacc.Bacc("TRN2", target_bir_lowering=False, debug=False,
                   num_devices=NCORES)

    xs_d = nc.dram_tensor("xs", [T, D], f32, kind="ExternalInput")
    ks_d = nc.dram_tensor("ks", [KM, D], f32r, kind="ExternalInput")
    vs_d = nc.dram_tensor("vs", [KM, D], f32r, kind="ExternalInput")
    out_d = nc.dram_tensor("out", [T, D], f32, kind="ExternalOutput")

    ks_r = ks_d.ap().rearrange("(c p) n -> c p n", p=P)   # [64, 128, 1024]
    vs_r = vs_d.ap().rearrange("(c p) n -> c p n", p=P)
    xs_r = xs_d.ap().rearrange("(c p) n -> c p n", p=P)   # [8, 128, 1024]

    with tile.TileContext(nc) as tc:
        with (
            tc.tile_pool(name="const", bufs=1) as const,
            tc.tile_pool(name="kfp", bufs=10) as kfp,
            tc.tile_pool(name="vfp", bufs=11) as vfp,
            tc.tile_pool(name="ktl", bufs=W) as ktl,
            tc.tile_pool(name="vtl", bufs=W) as vtl,
            tc.tile_pool(name="accp", bufs=2) as accp,
            tc.tile_pool(name="xstage", bufs=TCH) as xstage,
            tc.tile_pool(name="xtp", bufs=DB) as xtp,
            tc.tile_pool(name="kvio", bufs=2) as kvio,
            tc.tile_pool(name="outp", bufs=2) as outp,
            tc.tile_pool(name="ps", bufs=8, space="PSUM") as ps,
            tc.tile_pool(name="dram", bufs=8, space="DRAM") as dram,
        ):
            ident = const.tile([P, P], f32)
            make_identity(nc, ident)

            # Warm-up collective: arms the ncfw collective stream so the
            # first real AllReduce trigger doesn't pay the ~11us wake-up.
            warm = const.tile([P, 16], f16)
            nc.gpsimd.memset(warm[:], 0.0)
            warm_in = dram.tile([P, 16], f16, name="warm_in")
            warm_out = dram.tile([P, 16], f16, name="warm_out",
                                 addr_space="Shared")
            nc.gpsimd.dma_start(out=warm_in[:], in_=warm[:])
            nc.gpsimd.collective_compute(
                "AllReduce",
                ADD,
                replica_groups=[list(range(NCORES))],
                ins=[warm_in.opt()],
                outs=[warm_out.opt()],
            )

            # kv accumulator: acc[h][:, j*512:(j+1)*512] holds
            # kv[j*128:(j+1)*128, h*512:(h+1)*512] (fp16).  One tile per
            # half so the AR bounce/readback are single 1MiB DMAs.
            acc = [accp.tile([P, DB * HALF], f16, name=f"acc{h}",
                             tag="acc") for h in range(2)]
            for h in range(2):
                nc.vector.memset(acc[h][:], 0.0)

            # ---- stage 1 front: 48 chunks in groups of 8, both halves --
            # keys/vals stream on the sync queue; first chunks row-split
            # across two DMAs to shorten the start ramp.
            kf_tiles = [None] * NC_
            vf_tiles = [None] * NC_

            def load_chunk(c):
                kt = kfp.tile([P, D], f32r, name="kt", tag="kt")
                vt = vfp.tile([P, D], f32r, name="vt", tag="vt")
                if c < 2:
                    nc.sync.dma_start(out=kt[:, :HALF],
                                      in_=ks_r[c][:, :HALF])
                    nc.sync.dma_start(out=vt[:, :HALF],
                                      in_=vs_r[c][:, :HALF])
                    nc.sync.dma_start(out=kt[:, HALF:],
                                      in_=ks_r[c][:, HALF:])
                    nc.sync.dma_start(out=vt[:, HALF:],
                                      in_=vs_r[c][:, HALF:])
                else:
                    nc.sync.dma_start(out=kt[:], in_=ks_r[c])
                    nc.sync.dma_start(out=vt[:], in_=vs_r[c])
                kf_tiles[c] = kt
                vf_tiles[c] = vt

            c0 = 0
            for gi, gsz in enumerate(FRONT_GROUPS):
                for ci in range(gsz):
                    load_chunk(c0 + ci)
                for h in range(2):
                    e0 = h * HALF
                    pst = [ps.tile([P, HALF], f32, name=f"kv{h}_{j}",
                                   tag="ps") for j in range(DB)]
                    for ci in range(gsz):
                        c = c0 + ci
                        for j in range(DB):
                            nc.tensor.matmul(
                                pst[j][:],
                                kf_tiles[c][:, j * P:(j + 1) * P],
                                vf_tiles[c][:, e0:e0 + HALF],
                                start=(ci == 0), stop=(ci == gsz - 1))
                    for j in range(DB):
                        sl = slice(j * HALF, (j + 1) * HALF)
                        nc.vector.tensor_tensor(
                            out=acc[h][:, sl],
                            in0=pst[j][:],
                            in1=acc[h][:, sl],
                            op=ADD)
                c0 += gsz

            # Tail chunk loads continue the same stream.
            for c in range(FRONT, NC_):
                load_chunk(c)
            # x loads at the tail of the load stream.
            xf_tiles = []
            for i in range(TCH):
                xf = xstage.tile([P, D], f32, name="xf", tag="xf")
                nc.sync.dma_start(out=xf[:], in_=xs_r[i])
                xf_tiles.append(xf)

            # fp16 casts for the tail hold.  Emitted only now so they sit
            # behind all front drains in the Vector queue (their staging
            # DMAs land mid-stream; emitting them earlier head-blocks the
            # drains and stalls the PSUM pipeline).  keys (full) on
            # Vector, vals' second half on Scalar.
            kh_tiles = [None] * NC_
            vh_tiles = [None] * NC_
            for c in range(FRONT, NC_):
                kh = ktl.tile([P, D], f16, name="kh", tag="kh")
                nc.vector.tensor_copy(out=kh[:],
                                      in_=kf_tiles[c][:].bitcast(f32))
                kh_tiles[c] = kh
            for c in range(FRONT, NC_):
                vh = vtl.tile([P, HALF], f16, name="vh", tag="vh")
                nc.scalar.activation(vh[:],
                                     vf_tiles[c][:, HALF:].bitcast(f32),
                                     ACT_COPY)
                vh_tiles[c] = vh

            # ---- stage 1 tail, h=0 from f32r staging ----
            pst = [ps.tile([P, HALF], f32, name=f"kvt0_{j}", tag="ps")
                   for j in range(DB)]
            for ci in range(W):
                c = FRONT + ci
                for j in range(DB):
                    nc.tensor.matmul(
                        pst[j][:],
                        kf_tiles[c][:, j * P:(j + 1) * P],
                        vf_tiles[c][:, 0:HALF],
                        start=(ci == 0), stop=(ci == W - 1))
            for j in range(DB):
                sl = slice(j * HALF, (j + 1) * HALF)
                nc.vector.tensor_tensor(
                    out=acc[0][:, sl], in0=pst[j][:], in1=acc[0][:, sl],
                    op=ADD)

            # AllReduce h=0: DMA acc tiles straight into the bounce
            # buffer (fp16, no cast step).
            bounce_out = []
            for h in range(2):
                b_in = dram.tile([P, DB * HALF], f16,
                                 name=f"bin{h}", tag="bin")
                b_out = dram.tile([P, DB * HALF], f16,
                                  name=f"bout{h}", tag="bout",
                                  addr_space="Shared")
                bounce_out.append((b_in, b_out))

            def emit_ar(h):
                b_in, b_out = bounce_out[h]
                for j in range(DB):
                    sl = slice(j * HALF, (j + 1) * HALF)
                    nc.gpsimd.dma_start(out=b_in[:, sl], in_=acc[h][:, sl])
                nc.gpsimd.collective_compute(
                    "AllReduce",
                    ADD,
                    replica_groups=[list(range(NCORES))],
                    ins=[b_in.opt()],
                    outs=[b_out.opt()],
                )

            emit_ar(0)

            # ---- stage 1 tail, h=1 from the fp16 hold ----
            pst = [ps.tile([P, HALF], f32, name=f"kvt1_{j}", tag="ps")
                   for j in range(DB)]
            for ci in range(W):
                c = FRONT + ci
                for j in range(DB):
                    nc.tensor.matmul(
                        pst[j][:],
                        kh_tiles[c][:, j * P:(j + 1) * P],
                        vh_tiles[c][:],
                        start=(ci == 0), stop=(ci == W - 1))
            for j in range(DB):
                sl = slice(j * HALF, (j + 1) * HALF)
                nc.vector.tensor_tensor(
                    out=acc[1][:, sl], in0=pst[j][:], in1=acc[1][:, sl],
                    op=ADD)
            emit_ar(1)

            # ---- x: PE-transpose, cast to fp16 (fills AR wait) ----
            xT = [xtp.tile([P, T], f16, name=f"xT{j}", tag="xT")
                  for j in range(DB)]
            for i in range(TCH):
                xf = xf_tiles[i]
                for j in range(DB):
                    pst = ps.tile([P, P], f32, name="pst", tag="ps")
                    nc.tensor.transpose(
                        pst[:], xf[:, j * P:(j + 1) * P], ident[:])
                    nc.vector.tensor_copy(
                        out=xT[j][:, i * P:(i + 1) * P], in_=pst[:])

            # PE warmers: the AR waits leave the PE idle long enough
            # for HAM to re-throttle it to 1.2 GHz, which would slow all
            # of stage 2.  A chain of dummy matmuls (no data deps beyond
            # the constant tile) bridges the gap; stage-2 h=0 is hidden
            # under the h=1 AllReduce, so the ~8us of queue delay they
            # can add there is free.
            wsrc = const.tile([P, HALF], f32r)
            nc.vector.memset(wsrc[:].bitcast(f32), 0.0)

            def warmers(n):
                wps = ps.tile([P, HALF], f32, name="wps", tag="ps")
                for _ in range(n):
                    nc.tensor.matmul(wps[:], wsrc[:, :P], wsrc[:],
                                     start=True, stop=True)

            warmers(30)

            # ---- stage 2: out = x @ kv, per column half ----
            for h in range(2):
                kvh = kvio.tile([P, DB * HALF], f16, name=f"kvr{h}",
                                tag="kvio")
                nc.gpsimd.dma_start(out=kvh[:], in_=bounce_out[h][1][:])
                if h == 1:
                    warmers(40)
                for i in range(TCH):
                    po = ps.tile([P, HALF], f32, name="po", tag="ps")
                    for j in range(DB):
                        nc.tensor.matmul(
                            po[:],
                            xT[j][:, i * P:(i + 1) * P],
                            kvh[:, j * HALF:(j + 1) * HALF],
                            start=(j == 0), stop=(j == DB - 1))
                    ob = outp.tile([P, HALF], f32, name="ob", tag="ob")
                    nc.scalar.activation(ob[:], po[:], ACT_COPY)
                    nc.scalar.dma_start(
                        out=out_d.ap()[i * P:(i + 1) * P,
                                       h * HALF:(h + 1) * HALF],
                        in_=ob[:])

    nc.compile()
    return nc


def _get_nc():
    if "nc" not in _CACHE:
        _CACHE["nc"] = _build_nc()
    return _CACHE["nc"]


def kernel(**inputs):
    from concourse.bass_utils import run_bass_kernel_spmd

    x = np.ascontiguousarray(np.asarray(inputs["x"], dtype=np.float32))
    keys = np.ascontiguousarray(np.asarray(inputs["keys"], dtype=np.float32))
    vals = np.ascontiguousarray(np.asarray(inputs["vals"], dtype=np.float32))
    xf = x.reshape(B * S, D)

    nc = _get_nc()
    in_maps = []
    for c in range(NCORES):
        in_maps.append({
            "xs": xf[c * T:(c + 1) * T],
            "ks": keys[c * KM:(c + 1) * KM],
            "vs": vals[c * KM:(c + 1) * KM],
        })
    res = run_bass_kernel_spmd(nc, in_maps, list(range(NCORES)))
    out = np.concatenate([res.results[c]["out"] for c in range(NCORES)],
                         axis=0)
    return out.reshape(B, S, D).astype(np.float32)

